# revision 5
# baseline (speedup 1.0000x reference)
"""Trainium2 Bass kernel for nn_Encoder_7894149890238.

reference semantics (B=192, D=2048, H=128):
    mu  = relu-MLP_mu(q)   [B, D]
    lv  = relu-MLP_lv(q)   [B, D]
    var = exp(0.5*lv); scale = sqrt(var) = exp(0.25*lv)
    p[i, j, :]    = mu[j] + eps[i, j, :] * scale[j]            [B, B, D]
    maha[i, j]    = sum_d (p-mu)^2/var = sum_d eps[i, j, d]^2  [B, B]
    log_prob[i,j] = -0.5*(maha + D*log(2*pi)) - 0.25*sum_d lv[j, d]

The O(B^2 D) work (p and the eps^2 row-sums) runs on 8 NeuronCores,
data-parallel over the sample axis i (24 samples/core). The tiny MLPs
(~0.4 GFLOP) run on host and mu/scale are replicated to every core, per
the sharding hint. Per core the Bass kernel streams 36 tiles of
[128 rows, 2048] f32:

  - DMA-in on the SP HWDGE ring
  - p = eps*scale + mu: two tensor_tensor ops, on DVE for 2/3 of the
    tiles and GpSimd for 1/3 (GpSimd TT is ~2x DVE cost, so this
    balances both at ~110us/core, under the ~211us HBM roofline)
  - ssq = rowsum(eps^2): one ACT Square+accum_out op
  - DMA-out of p on the ACT HWDGE ring

Raw Bass with manual semaphores (TileContext's tail drain trips this
walrus build). DMA completion order is NOT FIFO across dma_starts
(SDMA engines round-robin queues at packet granularity), so every
DMA waits on a dedicated per-buffer-slot semaphore with at most one
DMA in flight per slot — the count then identifies the tile.

Row r of a core's flattened [4608, 2048] shard has j = r % 192, so a
128-row tile needs mu/scale rows (128*t + arange(128)) % 192 — periodic
in t with period 3. The 3 phase-banks for scale and mu are prebuilt on
host and kept in SBUF.
"""

import numpy as np

B = 192
D = 2048
LOG2PI = float(np.log(2.0 * np.pi))
N_CORES = 8
SHARD = B // N_CORES          # 24 samples per core
ROWS = SHARD * B              # 4608 rows per core
P = 128                       # partitions per tile
TILES = ROWS // P             # 36
PHASES = 3                    # lcm(128, 192)/128
NBUF = 4                      # eps/p buffer slots


def _gp_owned(t):
    """Tiles whose elementwise FMA runs on GpSimd instead of DVE."""
    return t % 3 == 2


def _build_bass():
    import concourse.bass as bass
    from concourse import mybir

    f32 = mybir.dt.float32
    nc = bass.Bass("TRN2", target_bir_lowering=False, num_devices=N_CORES)

    eps = nc.dram_tensor("eps", [ROWS, D], f32, kind="ExternalInput")
    banks = nc.dram_tensor("banks", [2 * PHASES, P, D], f32, kind="ExternalInput")
    p_out = nc.dram_tensor("p", [ROWS, D], f32, kind="ExternalOutput")
    ssq_out = nc.dram_tensor("ssq", [P, TILES], f32, kind="ExternalOutput")

    # per-engine op counters: nth_op[t] = index of tile t within its owner's
    # stream; compute-sem waits are owner-local (+2 per owned tile).
    own_idx = {}
    nv = ng = 0
    for t in range(TILES):
        if _gp_owned(t):
            own_idx[t] = ng
            ng += 1
        else:
            own_idx[t] = nv
            nv += 1

    import contextlib

    with contextlib.ExitStack() as ctx:
        em = ctx.enter_context
        bank_sem = [em(nc.semaphore(f"bk{k}")) for k in range(2 * PHASES)]
        in_b = [em(nc.semaphore(f"in_b{b}")) for b in range(NBUF)]
        out_b = [em(nc.semaphore(f"out_b{b}")) for b in range(NBUF)]
        v_sem = em(nc.semaphore("v_sem"))      # DVE ops, +1 each
        g_sem = em(nc.semaphore("g_sem"))      # GpSimd ops, +1 each
        a_sem = em(nc.semaphore("a_sem"))      # ACT squares, +1 each
        s_done = em(nc.semaphore("s_done"))    # final ssq store

        eps_buf = [em(nc.sbuf_tensor(f"eps{b}", [P, D], f32)) for b in range(NBUF)]
        p_buf = [em(nc.sbuf_tensor(f"pb{b}", [P, D], f32)) for b in range(NBUF)]
        sbank = [em(nc.sbuf_tensor(f"sb{k}", [P, D], f32)) for k in range(PHASES)]
        mbank = [em(nc.sbuf_tensor(f"mb{k}", [P, D], f32)) for k in range(PHASES)]
        sq = em(nc.sbuf_tensor("sq", [P, D], f32))
        ssq_sb = em(nc.sbuf_tensor("ssq_sb", [P, TILES], f32))

        def mult_done(t):        # owner sem value once mult of tile t ran
            return (v_sem if not _gp_owned(t) else g_sem), 2 * own_idx[t] + 1

        def add_done(t):
            return (v_sem if not _gp_owned(t) else g_sem), 2 * own_idx[t] + 2

        def compute_stream(engine, owned, sem):
            # banks needed first by this engine: phase of first owned tiles
            seen_phase = set()
            for t in range(TILES):
                if not owned(t):
                    continue
                b = t % NBUF
                k = t % PHASES
                if k not in seen_phase:
                    seen_phase.add(k)
                    engine.wait_ge(bank_sem[k], 16)            # sbank[k]
                    engine.wait_ge(bank_sem[PHASES + k], 16)   # mbank[k]
                engine.wait_ge(in_b[b], 16 * (t // NBUF + 1))
                if t >= NBUF:
                    # p_buf[b]'s previous DMA-out (tile t-NBUF) must be done
                    engine.wait_ge(out_b[b], 16 * (t // NBUF))
                engine.tensor_mul(
                    p_buf[b].ap(), eps_buf[b].ap(), sbank[k].ap()
                ).then_inc(sem, 1)
                engine.tensor_add(
                    p_buf[b].ap(), p_buf[b].ap(), mbank[k].ap()
                ).then_inc(sem, 1)

        with nc.Block() as block:

            @block.sync
            def _(sync):
                def in_dma(t):
                    b = t % NBUF
                    if t >= NBUF:
                        tp = t - NBUF  # previous tile in this slot
                        sem, val = mult_done(tp)
                        sync.wait_ge(sem, val)
                        sync.wait_ge(a_sem, tp + 1)
                    sync.dma_start(
                        eps_buf[b].ap(), eps.ap()[t * P : (t + 1) * P, :]
                    ).then_inc(in_b[b], 16)

                def bank_dma(k):
                    tgt = sbank[k] if k < PHASES else mbank[k - PHASES]
                    sync.dma_start(tgt.ap(), banks.ap()[k]).then_inc(bank_sem[k], 16)

                # prologue: first tile + phase-0 banks first, prefetch the
                # rest of the buffers, then the remaining banks
                in_dma(0)
                bank_dma(0)            # sbank 0
                bank_dma(PHASES)       # mbank 0
                for t in range(1, NBUF):
                    in_dma(t)
                for k in (1, 2):
                    bank_dma(k)
                    bank_dma(PHASES + k)
                for t in range(NBUF, TILES):
                    in_dma(t)
                for b in range(NBUF):
                    n_stores = len([t for t in range(TILES) if t % NBUF == b])
                    sync.wait_ge(out_b[b], 16 * n_stores)
                sync.wait_ge(s_done, 16)

            @block.vector
            def _(vector):
                compute_stream(vector, lambda t: not _gp_owned(t), v_sem)

            @block.gpsimd
            def _(gpsimd):
                compute_stream(gpsimd, _gp_owned, g_sem)

            @block.scalar
            def _(scalar):
                from concourse import mybir as _mb

                for t in range(TILES):
                    b = t % NBUF
                    scalar.wait_ge(in_b[b], 16 * (t // NBUF + 1))
                    scalar.activation(
                        sq.ap(),
                        eps_buf[b].ap(),
                        _mb.ActivationFunctionType.Square,
                        accum_out=ssq_sb.ap()[:, t : t + 1],
                    ).then_inc(a_sem, 1)
                    sem, val = add_done(t)
                    scalar.wait_ge(sem, val)
                    scalar.dma_start(
                        p_out.ap()[t * P : (t + 1) * P, :], p_buf[b].ap()
                    ).then_inc(out_b[b], 16)
                scalar.dma_start(ssq_out.ap(), ssq_sb.ap()).then_inc(s_done, 16)

    return nc


_NC_CACHE = None


def _get_nc():
    global _NC_CACHE
    if _NC_CACHE is None:
        _NC_CACHE = _build_bass()
    return _NC_CACHE


def _host_heads(q, w):
    """mu, lv via the tiny MLPs in f32 (replicated, computed once on host)."""
    relu = lambda a: np.maximum(a, 0.0)

    def head(w1, b1, w2, b2, w3, b3):
        h = relu(q @ w1.T + b1)
        h = relu(h @ w2.T + b2)
        return relu(h @ w3.T + b3)

    mu = head(w["mu_w1"], w["mu_b1"], w["mu_w2"], w["mu_b2"], w["mu_w3"], w["mu_b3"])
    lv = head(w["lv_w1"], w["lv_b1"], w["lv_w2"], w["lv_b2"], w["lv_w3"], w["lv_b3"])
    return mu.astype(np.float32), lv.astype(np.float32)


def _run(inputs, trace=False, tmpdir=None):
    from concourse.bass_utils import run_bass_kernel_spmd

    f32 = np.float32
    q = np.asarray(inputs["q"], dtype=f32)
    eps = np.asarray(inputs["eps"], dtype=f32)
    w = {k: np.asarray(v, dtype=f32) for k, v in inputs.items() if k not in ("q", "eps")}

    mu, lv = _host_heads(q, w)
    var = np.exp(np.float32(0.5) * lv)
    scale = np.sqrt(var)

    # phase banks: tile phase k needs rows (128*k + arange(128)) % 192
    banks = np.empty((2 * PHASES, P, D), dtype=f32)
    for k in range(PHASES):
        idx = (P * k + np.arange(P)) % B
        banks[k] = scale[idx]
        banks[PHASES + k] = mu[idx]

    in_maps = [
        {
            "eps": np.ascontiguousarray(
                eps[c * SHARD : (c + 1) * SHARD].reshape(ROWS, D)
            ),
            "banks": banks,
        }
        for c in range(N_CORES)
    ]

    nc = _get_nc()
    res = run_bass_kernel_spmd(
        nc,
        in_maps,
        core_ids=list(range(N_CORES)),
        trace=trace,
        tmpdir=tmpdir,
    )

    p_full = np.empty((B, B, D), dtype=f32)
    ssq = np.empty((B, B), dtype=f32)
    for c in range(N_CORES):
        p_full[c * SHARD : (c + 1) * SHARD] = res.results[c]["p"].reshape(SHARD, B, D)
        ssq[c * SHARD : (c + 1) * SHARD] = res.results[c]["ssq"].T.reshape(SHARD, B)

    logdet_half = np.float32(0.25) * lv.sum(axis=1, dtype=f32)  # 0.5 * logdet
    log_prob = (
        np.float32(-0.5) * (ssq + np.float32(D * LOG2PI)) - logdet_half[None, :]
    ).astype(f32)
    return (p_full, log_prob), res


def kernel(**inputs):
    (p_full, log_prob), _ = _run(inputs, trace=False)
    return p_full, log_prob


# revision 6
# speedup vs baseline: 1.2031x; 1.2031x over previous
"""Trainium2 Bass kernel for nn_Encoder_7894149890238.

reference semantics (B=192, D=2048, H=128):
    mu  = relu-MLP_mu(q)   [B, D]
    lv  = relu-MLP_lv(q)   [B, D]
    var = exp(0.5*lv); scale = sqrt(var) = exp(0.25*lv)
    p[i, j, :]    = mu[j] + eps[i, j, :] * scale[j]            [B, B, D]
    maha[i, j]    = sum_d (p-mu)^2/var = sum_d eps[i, j, d]^2  [B, B]
    log_prob[i,j] = -0.5*(maha + D*log(2*pi)) - 0.25*sum_d lv[j, d]

The O(B^2 D) work (p and the eps^2 row-sums) runs on 8 NeuronCores,
data-parallel over the sample axis i (24 samples/core). The tiny MLPs
(~0.4 GFLOP) run on host and mu/scale are replicated to every core, per
the sharding hint. Per core the Bass kernel streams 36 tiles of
[128 rows, 2048] f32:

  - DMA-in on the SP HWDGE ring
  - p = eps*scale + mu: two tensor_tensor ops, on DVE for 2/3 of the
    tiles and GpSimd for 1/3 (GpSimd TT is ~2x DVE cost, so this
    balances both at ~110us/core, under the ~211us HBM roofline)
  - ssq = rowsum(eps^2): one ACT Square+accum_out op
  - DMA-out of p on the ACT HWDGE ring

Raw Bass with manual semaphores (TileContext's tail drain trips this
walrus build). DMA completion order is NOT FIFO across dma_starts
(SDMA engines round-robin queues at packet granularity), so every
DMA waits on a dedicated per-buffer-slot semaphore with at most one
DMA in flight per slot — the count then identifies the tile.

Row r of a core's flattened [4608, 2048] shard has j = r % 192, so a
128-row tile needs mu/scale rows (128*t + arange(128)) % 192 — periodic
in t with period 3. The 3 phase-banks for scale and mu are prebuilt on
host and kept in SBUF.
"""

import numpy as np

B = 192
D = 2048
LOG2PI = float(np.log(2.0 * np.pi))
N_CORES = 8
SHARD = B // N_CORES          # 24 samples per core
ROWS = SHARD * B              # 4608 rows per core
P = 128                       # partitions per tile
TILES = ROWS // P             # 36
PHASES = 3                    # lcm(128, 192)/128
NBUF = 4                      # eps/p buffer slots


def _gp_owned(t):
    """Tiles whose elementwise FMA runs on GpSimd instead of DVE.
    Measured: concurrent GpSimd elementwise work contends with DVE for
    SBUF ports and slows DVE TTs ~2x (2.29us -> 4.43us), a net loss —
    so everything stays on DVE."""
    return False


def _build_bass():
    import concourse.bass as bass
    from concourse import mybir

    f32 = mybir.dt.float32
    nc = bass.Bass("TRN2", target_bir_lowering=False, num_devices=N_CORES)

    eps = nc.dram_tensor("eps", [ROWS, D], f32, kind="ExternalInput")
    banks = nc.dram_tensor("banks", [2 * PHASES, P, D], f32, kind="ExternalInput")
    p_out = nc.dram_tensor("p", [ROWS, D], f32, kind="ExternalOutput")
    ssq_out = nc.dram_tensor("ssq", [P, TILES], f32, kind="ExternalOutput")

    # per-engine op counters: nth_op[t] = index of tile t within its owner's
    # stream; compute-sem waits are owner-local (+2 per owned tile).
    own_idx = {}
    nv = ng = 0
    for t in range(TILES):
        if _gp_owned(t):
            own_idx[t] = ng
            ng += 1
        else:
            own_idx[t] = nv
            nv += 1

    import contextlib

    with contextlib.ExitStack() as ctx:
        em = ctx.enter_context
        bank_sem = [em(nc.semaphore(f"bk{k}")) for k in range(2 * PHASES)]
        in_b = [em(nc.semaphore(f"in_b{b}")) for b in range(NBUF)]
        out_b = [em(nc.semaphore(f"out_b{b}")) for b in range(NBUF)]
        v_sem = em(nc.semaphore("v_sem"))      # DVE ops, +1 each
        g_sem = em(nc.semaphore("g_sem"))      # GpSimd ops, +1 each
        a_sem = em(nc.semaphore("a_sem"))      # ACT squares, +1 each
        s_done = em(nc.semaphore("s_done"))    # final ssq store

        eps_buf = [em(nc.sbuf_tensor(f"eps{b}", [P, D], f32)) for b in range(NBUF)]
        p_buf = [em(nc.sbuf_tensor(f"pb{b}", [P, D], f32)) for b in range(NBUF)]
        sbank = [em(nc.sbuf_tensor(f"sb{k}", [P, D], f32)) for k in range(PHASES)]
        mbank = [em(nc.sbuf_tensor(f"mb{k}", [P, D], f32)) for k in range(PHASES)]
        sq = em(nc.sbuf_tensor("sq", [P, D], f32))
        ssq_sb = em(nc.sbuf_tensor("ssq_sb", [P, TILES], f32))

        def mult_done(t):        # owner sem value once mult of tile t ran
            return (v_sem if not _gp_owned(t) else g_sem), 2 * own_idx[t] + 1

        def add_done(t):
            return (v_sem if not _gp_owned(t) else g_sem), 2 * own_idx[t] + 2

        def compute_stream(engine, owned, sem):
            # banks needed first by this engine: phase of first owned tiles
            seen_phase = set()
            for t in range(TILES):
                if not owned(t):
                    continue
                b = t % NBUF
                k = t % PHASES
                if k not in seen_phase:
                    seen_phase.add(k)
                    engine.wait_ge(bank_sem[k], 16)            # sbank[k]
                    engine.wait_ge(bank_sem[PHASES + k], 16)   # mbank[k]
                engine.wait_ge(in_b[b], 16 * (t // NBUF + 1))
                if t >= NBUF:
                    # p_buf[b]'s previous DMA-out (tile t-NBUF) must be done
                    engine.wait_ge(out_b[b], 16 * (t // NBUF))
                engine.tensor_mul(
                    p_buf[b].ap(), eps_buf[b].ap(), sbank[k].ap()
                ).then_inc(sem, 1)
                engine.tensor_add(
                    p_buf[b].ap(), p_buf[b].ap(), mbank[k].ap()
                ).then_inc(sem, 1)

        with nc.Block() as block:

            @block.sync
            def _(sync):
                def in_dma(t):
                    b = t % NBUF
                    if t >= NBUF:
                        tp = t - NBUF  # previous tile in this slot
                        sem, val = mult_done(tp)
                        sync.wait_ge(sem, val)
                        sync.wait_ge(a_sem, tp + 1)
                    sync.dma_start(
                        eps_buf[b].ap(), eps.ap()[t * P : (t + 1) * P, :]
                    ).then_inc(in_b[b], 16)

                def bank_dma(k):
                    tgt = sbank[k] if k < PHASES else mbank[k - PHASES]
                    sync.dma_start(tgt.ap(), banks.ap()[k]).then_inc(bank_sem[k], 16)

                # prologue: first tile + phase-0 banks first, prefetch the
                # rest of the buffers, then the remaining banks
                in_dma(0)
                bank_dma(0)            # sbank 0
                bank_dma(PHASES)       # mbank 0
                for t in range(1, NBUF):
                    in_dma(t)
                for k in (1, 2):
                    bank_dma(k)
                    bank_dma(PHASES + k)
                for t in range(NBUF, TILES):
                    in_dma(t)
                for b in range(NBUF):
                    n_stores = len([t for t in range(TILES) if t % NBUF == b])
                    sync.wait_ge(out_b[b], 16 * n_stores)
                sync.wait_ge(s_done, 16)

            @block.vector
            def _(vector):
                compute_stream(vector, lambda t: not _gp_owned(t), v_sem)

            @block.gpsimd
            def _(gpsimd):
                compute_stream(gpsimd, _gp_owned, g_sem)

            @block.scalar
            def _(scalar):
                from concourse import mybir as _mb

                for t in range(TILES):
                    b = t % NBUF
                    scalar.wait_ge(in_b[b], 16 * (t // NBUF + 1))
                    scalar.activation(
                        sq.ap(),
                        eps_buf[b].ap(),
                        _mb.ActivationFunctionType.Square,
                        accum_out=ssq_sb.ap()[:, t : t + 1],
                    ).then_inc(a_sem, 1)
                    sem, val = add_done(t)
                    scalar.wait_ge(sem, val)
                    scalar.dma_start(
                        p_out.ap()[t * P : (t + 1) * P, :], p_buf[b].ap()
                    ).then_inc(out_b[b], 16)
                scalar.dma_start(ssq_out.ap(), ssq_sb.ap()).then_inc(s_done, 16)

    return nc


_NC_CACHE = None


def _get_nc():
    global _NC_CACHE
    if _NC_CACHE is None:
        _NC_CACHE = _build_bass()
    return _NC_CACHE


def _host_heads(q, w):
    """mu, lv via the tiny MLPs in f32 (replicated, computed once on host)."""
    relu = lambda a: np.maximum(a, 0.0)

    def head(w1, b1, w2, b2, w3, b3):
        h = relu(q @ w1.T + b1)
        h = relu(h @ w2.T + b2)
        return relu(h @ w3.T + b3)

    mu = head(w["mu_w1"], w["mu_b1"], w["mu_w2"], w["mu_b2"], w["mu_w3"], w["mu_b3"])
    lv = head(w["lv_w1"], w["lv_b1"], w["lv_w2"], w["lv_b2"], w["lv_w3"], w["lv_b3"])
    return mu.astype(np.float32), lv.astype(np.float32)


def _run(inputs, trace=False, tmpdir=None):
    from concourse.bass_utils import run_bass_kernel_spmd

    f32 = np.float32
    q = np.asarray(inputs["q"], dtype=f32)
    eps = np.asarray(inputs["eps"], dtype=f32)
    w = {k: np.asarray(v, dtype=f32) for k, v in inputs.items() if k not in ("q", "eps")}

    mu, lv = _host_heads(q, w)
    var = np.exp(np.float32(0.5) * lv)
    scale = np.sqrt(var)

    # phase banks: tile phase k needs rows (128*k + arange(128)) % 192
    banks = np.empty((2 * PHASES, P, D), dtype=f32)
    for k in range(PHASES):
        idx = (P * k + np.arange(P)) % B
        banks[k] = scale[idx]
        banks[PHASES + k] = mu[idx]

    in_maps = [
        {
            "eps": np.ascontiguousarray(
                eps[c * SHARD : (c + 1) * SHARD].reshape(ROWS, D)
            ),
            "banks": banks,
        }
        for c in range(N_CORES)
    ]

    nc = _get_nc()
    res = run_bass_kernel_spmd(
        nc,
        in_maps,
        core_ids=list(range(N_CORES)),
        trace=trace,
        tmpdir=tmpdir,
    )

    p_full = np.empty((B, B, D), dtype=f32)
    ssq = np.empty((B, B), dtype=f32)
    for c in range(N_CORES):
        p_full[c * SHARD : (c + 1) * SHARD] = res.results[c]["p"].reshape(SHARD, B, D)
        ssq[c * SHARD : (c + 1) * SHARD] = res.results[c]["ssq"].T.reshape(SHARD, B)

    logdet_half = np.float32(0.25) * lv.sum(axis=1, dtype=f32)  # 0.5 * logdet
    log_prob = (
        np.float32(-0.5) * (ssq + np.float32(D * LOG2PI)) - logdet_half[None, :]
    ).astype(f32)
    return (p_full, log_prob), res


def kernel(**inputs):
    (p_full, log_prob), _ = _run(inputs, trace=False)
    return p_full, log_prob
